# revision 29
# baseline (speedup 1.0000x reference)
"""Trainium2 Bass kernel for nn_CFCML_20083267076887 (4-direction Mamba-style
selective-scan block between two 1x1 conv+BN+ReLU stages).

Sharding: 8 cores = (batch b in {0,1}) x (scan direction d in {0..3}).
 - channel flips (dirs 1,3) fold into w_in rows / w_out cols on host
 - L flips (dirs 2,3) feed the core a host-flipped x slice; host unflips the
   core's y output before the combine stage (pure data movement)

NEFF1 (per core) v2 layout: the selective scan runs with SBUF partitions
holding (state n, channel-sub ds) pairs: partition = n*8 + ds, over 16
channel groups g (d = g*8 + ds).  Per chunk [*, LC]:
 - GEMMs in fp32r (4x faster than fp32 on PE) for conv1/w_in/conv taps/
   x-proj/dt/w_out
 - softplus and silu exactly via the natural_log_exp ACT set:
   softplus(x) = Ln(Exp(x)+1); silu(x) = x * Exp(-Ln(Exp(-x)+1))
 - B/C rows broadcast once per chunk into (n,ds) layout via selector
   matmuls (SEL_B/SEL_C)
 - per group g: PE replicates delta/wdx rows via selector SELG_g; ACT
   computes dA = exp(A * delta_rep) from PSUM; DVE computes dBu, the
   tensor_tensor_scan (bf16 h out), and p = h*Cb (bf16 2x mode); PE
   accumulates y over the 16 groups in PSUM via 0/1 matmuls ONESG_g,
   plus D*xm via a diagonal matmul
 - y gated by silu(z) then projected by w_out

NEFF2 (per core = (b, quarter)): sum of 4 direction y quarters + residual
act_x (recomputed) -> 1x1 conv2 + BN + ReLU -> out slice [64, L/4]
"""
import sys
import numpy as np

for _p in ("/opt/trn_rl_repo", "/root/.axon_site/_ro/trn_rl_repo"):
    if _p not in sys.path:
        sys.path.append(_p)

import jax
from jax.sharding import Mesh, PartitionSpec
from jax.experimental.shard_map import shard_map

import concourse.bacc as bacc
import concourse.tile as tile
import concourse.mybir as mybir
from concourse import bass2jax

# Pin every ACTIVATE to the natural_log_exp_and_others table set (it holds
# exp/ln/relu/copy — everything these NEFFs use).  The stock per-function
# greedy choice alternates exp_and_others <-> natural_log on every Exp<->Ln
# boundary, inserting ~73 ACT_TABLE_LOADs (~94us) per NEFF1 run.  Emptying
# the other sets (ids preserved) makes the fixpoint hoist a single load.
import contextlib
import concourse.hw_specs as _hw_specs
_PIN_SET = "natural_log_exp_and_others"


@contextlib.contextmanager
def _pinned_act_set():
    orig = _hw_specs.get_activation_tables

    def _pinned(module_arch):
        tabs = orig(module_arch)
        return {name: (fns if name == _PIN_SET else set())
                for name, fns in tabs.items()}

    _hw_specs.get_activation_tables = _pinned
    bacc.get_activation_tables = _pinned
    try:
        yield
    finally:
        _hw_specs.get_activation_tables = orig
        bacc.get_activation_tables = orig

F32 = mybir.dt.float32
F32R = mybir.dt.float32r
BF16 = mybir.dt.bfloat16
AF = mybir.ActivationFunctionType
OP = mybir.AluOpType
ml_bf16 = mybir.dt.np(BF16)

B, C, DZ, H, W = 2, 64, 12, 32, 32
N = 16
DCONV = 4
DIN = 128
DTR = 4
L = DZ * H * W          # 12288
LQ = L // 4             # 3072
BN_EPS = 1e-5
LC = 1024
NCH = L // LC
NG = 16                 # channel groups of 8
DS = DIN // NG          # 8
N_CORES = 8


# ---------------------------------------------------------------- NEFF 1
def _build_neff1():
    nc = bacc.Bacc("TRN2", target_bir_lowering=False, debug=False,
                   num_devices=N_CORES)
    din = {}
    for name, shape, dt in [
        ("xb", [C, L], F32R),
        ("nin_wT", [C, C], F32R), ("s1", [C, 1], F32), ("t1", [C, 1], F32),
        ("w_in_x", [C, DIN], F32R), ("w_in_z", [C, DIN], F32R),
        ("conv_diag", [DIN, DCONV * DIN], F32R), ("conv_b", [DIN, 1], F32),
        ("conv_bn", [DIN, 1], F32),
        ("w_xproj", [DIN, DTR + 2 * N], F32R),
        ("w_dt", [DTR, DIN], F32R), ("b_dt", [DIN, 1], F32),
        ("A_perm", [DIN, NG], F32), ("D_diag", [DIN, DIN], F32R),
        ("w_out_q", [DIN, C], F32R),
        ("SEL_B", [DTR + 2 * N, DIN], F32R), ("SEL_C", [DTR + 2 * N, DIN], F32R),
        ("SELG", [DIN, NG * DIN], F32R), ("ONESG", [DIN, NG * DIN], BF16),
    ]:
        din[name] = nc.dram_tensor(name, shape, dt, kind="ExternalInput").ap()
    y_out = nc.dram_tensor("y_dir", [C, L], F32, kind="ExternalOutput").ap()

    def mm(ps, lhsT, rhs, start=True, stop=True):
        fd = rhs.shape[-1]
        for s0 in range(0, fd, 512):
            s1 = min(s0 + 512, fd)
            nc.tensor.matmul(ps[:, s0:s1], lhsT, rhs[:, s0:s1],
                             start=start, stop=stop)

    def mm_acc(ps, parts):
        fd = parts[0][1].shape[-1]
        for s0 in range(0, fd, 512):
            s1 = min(s0 + 512, fd)
            for k, (lhsT, rhs) in enumerate(parts):
                nc.tensor.matmul(ps[:, s0:s1], lhsT, rhs[:, s0:s1],
                                 start=(k == 0), stop=(k == len(parts) - 1))

    SL = 512
    NSL = LC // SL

    from contextlib import ExitStack
    with tile.TileContext(nc) as tc, ExitStack() as es:
        wp = es.enter_context(tc.tile_pool(name="wp", bufs=1))
        sp = es.enter_context(tc.tile_pool(name="sp", bufs=2))
        gp = es.enter_context(tc.tile_pool(name="gp", bufs=3))
        tp = es.enter_context(tc.tile_pool(name="tp", bufs=4))
        pp = es.enter_context(tc.tile_pool(name="pp", bufs=1))
        ap = es.enter_context(tc.tile_pool(name="ap", bufs=4))
        psG = es.enter_context(tc.tile_pool(name="psG", bufs=2, space="PSUM"))
        psY = es.enter_context(tc.tile_pool(name="psY", bufs=1, space="PSUM"))
        psR = es.enter_context(tc.tile_pool(name="psR", bufs=2, space="PSUM"))

        w = {}
        for name in din:
            if name == "xb":
                continue
            t = wp.tile(list(din[name].shape), din[name].tensor.dtype,
                        name=f"w_{name}")
            nc.sync.dma_start(t, din[name])
            w[name] = t
        carry = wp.tile([DIN, NG], F32, name="carry")

        # ~5us of dense matmuls at NEFF start: ramps the PE HAM clock-gate
        # to 8/8 so the steady-state mms run at 2.4 GHz from the first chunk
        pswu = psG.tile([DIN, 512], F32, name="pswu", tag="psG")
        for _ in range(12):
            nc.tensor.matmul(pswu, w["SELG"][:, 0:DIN],
                             w["SELG"][:, 0:512], start=True, stop=True)

        # ---------------- pipelined emission ----------------
        # emit_gemm_stages(ch) returns 4 closures (S0..S3) that produce the
        # chunk's GEMM-phase tensors; they are interleaved into the PREVIOUS
        # chunk's group loop so PE/ACT/DVE queues stay dense across chunks.
        states = {}

        def emit_gemm_stages(ch):
            st = {}
            states[ch] = st
            lo = ch * LC

            def S0():
                x_t = sp.tile([C, LC], F32R, name="x_t", tag="x_t")
                nc.sync.dma_start(x_t, din["xb"][:, lo:lo + LC])
                act = sp.tile([C, LC], F32R, name="act", tag="act")
                for s in range(NSL):
                    sl = slice(s * SL, (s + 1) * SL)
                    ps = psG.tile([C, SL], F32, name="ps_h0", tag="psG")
                    nc.tensor.matmul(ps, w["nin_wT"], x_t[:, sl],
                                     start=True, stop=True)
                    nc.scalar.activation(act[:, sl], ps, AF.Relu,
                                         scale=w["s1"][:, 0:1],
                                         bias=w["t1"][:, 0:1])
                xmpre = sp.tile([DIN, LC + 3], F32R, name="xmpre", tag="xmpre")
                for s in range(NSL):
                    sl = slice(s * SL, (s + 1) * SL)
                    ps = psG.tile([DIN, SL], F32, name="ps_xx", tag="psG")
                    nc.tensor.matmul(ps, w["w_in_x"], act[:, sl],
                                     start=True, stop=True)
                    nc.scalar.copy(xmpre[:, 3 + s * SL:3 + (s + 1) * SL], ps)
                if ch == 0:
                    nc.vector.memset(xmpre[:, 0:3].bitcast(F32), 0.0)
                else:
                    prev = states[ch - 1]["xmpre"]
                    nc.scalar.copy(xmpre[:, 0:3], prev[:, LC:LC + 3])
                st["act"], st["xmpre"] = act, xmpre

            def S1():
                act, xmpre = st["act"], st["xmpre"]
                xm = sp.tile([DIN, LC], F32R, name="xm", tag="xm")
                for s in range(NSL):
                    sl = slice(s * SL, (s + 1) * SL)
                    psc = psG.tile([DIN, SL], F32, name="ps_xc", tag="psG")
                    for k in range(DCONV):
                        nc.tensor.matmul(
                            psc, w["conv_diag"][:, k * DIN:(k + 1) * DIN],
                            xmpre[:, k + s * SL:k + s * SL + SL],
                            start=(k == 0), stop=(k == DCONV - 1))
                    s1_ = tp.tile([DIN, SL], F32, name="s1c", tag="tmp")
                    nc.scalar.activation(s1_, psc, AF.Exp, scale=-1.0,
                                         bias=w["conv_bn"][:, 0:1])
                    s2_ = tp.tile([DIN, SL], F32, name="s2c", tag="tmp")
                    nc.scalar.activation(s2_, s1_, AF.Ln, bias=1.0)
                    sg_ = tp.tile([DIN, SL], F32, name="sgc", tag="tmp")
                    nc.scalar.activation(sg_, s2_, AF.Exp, scale=-1.0)
                    nc.vector.scalar_tensor_tensor(
                        xm[:, sl], psc, w["conv_b"][:, 0:1], sg_,
                        OP.add, OP.mult)
                st["xm"] = xm

            def S2():
                xm = st["xm"]
                NBC = DTR + 2 * N
                bcS = sp.tile([NBC, LC], F32R, name="bcS", tag="bcS")
                dwx = sp.tile([DIN, 2 * LC], F32R, name="dwx", tag="dwx")
                delta = dwx[:, 0:LC]
                for s in range(NSL):
                    sl = slice(s * SL, (s + 1) * SL)
                    psbc = psG.tile([NBC, SL], F32, name="ps_bc", tag="psG")
                    nc.tensor.matmul(psbc, w["w_xproj"], xm[:, sl],
                                     start=True, stop=True)
                    nc.scalar.copy(bcS[:, sl], psbc)
                for s in range(NSL):
                    sl = slice(s * SL, (s + 1) * SL)
                    psp = psG.tile([DIN, SL], F32, name="ps_dpre", tag="psG")
                    nc.tensor.matmul(psp, w["w_dt"], bcS[0:DTR, sl],
                                     start=True, stop=True)
                    e_ = tp.tile([DIN, SL], F32, name="e_", tag="tmp")
                    nc.scalar.activation(e_, psp, AF.Exp,
                                         bias=w["b_dt"][:, 0:1])
                    nc.scalar.activation(delta[:, sl], e_, AF.Ln, bias=1.0)
                nc.vector.tensor_tensor(dwx[:, LC:2 * LC], delta, xm, OP.mult)
                st["dwx"], st["bcS"] = dwx, bcS

            def S3():
                act = st["act"]
                zs = sp.tile([DIN, LC], F32, name="zs", tag="zs")
                for s in range(NSL):
                    sl = slice(s * SL, (s + 1) * SL)
                    psz = psG.tile([DIN, SL], F32, name="ps_z", tag="psG")
                    nc.tensor.matmul(psz, w["w_in_z"], act[:, sl],
                                     start=True, stop=True)
                    z1 = tp.tile([DIN, SL], F32, name="z1", tag="tmp")
                    nc.scalar.activation(z1, psz, AF.Exp, scale=-1.0)
                    z2 = tp.tile([DIN, SL], F32, name="z2", tag="tmp")
                    nc.scalar.activation(z2, z1, AF.Ln, bias=1.0)
                    z3 = tp.tile([DIN, SL], F32, name="z3", tag="tmp")
                    nc.scalar.activation(z3, z2, AF.Exp, scale=-1.0)
                    nc.vector.tensor_tensor(zs[:, sl], psz, z3, OP.mult)
                st["zs"] = zs

            return [S0, S1, S2, S3]

        def emit_bbcb(st):
            bcS = st["bcS"]
            psB = psR.tile([DIN, LC], F32, name="psB", tag="psR")
            mm(psB, w["SEL_B"], bcS)
            Bb = sp.tile([DIN, LC], BF16, name="Bb", tag="Bb")
            nc.scalar.copy(Bb, psB)
            psC = psR.tile([DIN, LC], F32, name="psC", tag="psR")
            mm(psC, w["SEL_C"], bcS)
            Cb = sp.tile([DIN, LC], BF16, name="Cb", tag="Cb")
            nc.scalar.copy(Cb, psC)
            st["Bb"], st["Cb"] = Bb, Cb

        # prologue: chunk 0 GEMM phase + Bb/Cb
        for f in emit_gemm_stages(0):
            f()
        emit_bbcb(states[0])

        for ch in range(NCH):
            st = states[ch]
            delta = st["dwx"][:, 0:LC]
            wdx = st["dwx"][:, LC:2 * LC]
            nxt = emit_gemm_stages(ch + 1) if ch + 1 < NCH else []
            psy = psY.tile([DIN, LC], F32, name="psy", tag="psy")
            p_tiles = {}
            for g in range(NG):
                sel_g = w["SELG"][:, g * DIN:(g + 1) * DIN]
                psdD = psR.tile([DIN, LC], F32, name="psdD", tag="psR")
                mm(psdD, sel_g, delta)
                dA = ap.tile([DIN, LC], F32, name="dA", tag="dA")
                nc.scalar.activation(dA, psdD, AF.Exp,
                                     scale=w["A_perm"][:, g:g + 1])
                psdW = psR.tile([DIN, LC], F32, name="psdW", tag="psR")
                mm(psdW, sel_g, wdx)
                wdxR = ap.tile([DIN, LC], BF16, name="wdxR", tag="wdxR")
                nc.scalar.copy(wdxR, psdW)
                dBu = ap.tile([DIN, LC], BF16, name="dBu", tag="dBu")
                nc.vector.tensor_tensor(dBu, wdxR, st["Bb"], OP.mult)
                h = gp.tile([DIN, LC], BF16, name="h", tag="h")
                init = 0.0 if ch == 0 else carry[:, g:g + 1]
                nc.vector.tensor_tensor_scan(h, dA, dBu, init,
                                             OP.mult, OP.add)
                nc.scalar.copy(carry[:, g:g + 1], h[:, LC - 1:LC])
                p = gp.tile([DIN, LC], BF16, name="p", tag="p")
                nc.vector.tensor_tensor(p, h, st["Cb"], OP.mult)
                p_tiles[g] = p
                # y-acc for the previous group (keeps PE stream dense)
                if g >= 1:
                    mm(psy, w["ONESG"][:, (g - 1) * DIN:g * DIN],
                       p_tiles.pop(g - 1), start=(g == 1), stop=False)
                # interleave next chunk's GEMM stages
                if nxt:
                    if g == 4:
                        nxt[0]()
                    elif g == 7:
                        nxt[1]()
                    elif g == 10:
                        nxt[2]()
                    elif g == 13:
                        nxt[3]()
            mm(psy, w["ONESG"][:, (NG - 1) * DIN:NG * DIN],
               p_tiles.pop(NG - 1), start=False, stop=False)
            # + D * xm (diagonal matmul closes the accumulation group)
            mm(psy, w["D_diag"], st["xm"], start=False, stop=True)

            # y = psy * zs ; project by w_out
            y = sp.tile([DIN, LC], F32R, name="y", tag="y")
            nc.vector.tensor_tensor(y, psy, st["zs"], OP.mult)
            yo = sp.tile([C, LC], F32, name="yo", tag="yo")
            for s in range(NSL):
                sl = slice(s * SL, (s + 1) * SL)
                pso = psG.tile([C, SL], F32, name="ps_yo", tag="psG")
                nc.tensor.matmul(pso, w["w_out_q"], y[:, sl],
                                 start=True, stop=True)
                nc.scalar.copy(yo[:, sl], pso)
            nc.sync.dma_start(y_out[:, ch * LC:(ch + 1) * LC], yo)
            if nxt:
                emit_bbcb(states[ch + 1])

    nc.compile()
    return nc


# ---------------------------------------------------------------- NEFF 2
def _build_neff2():
    nc = bacc.Bacc("TRN2", target_bir_lowering=False, debug=False,
                   num_devices=N_CORES)
    din = {}
    for name, shape, dt in [
        ("yq0", [C, LQ], F32), ("yq1", [C, LQ], F32), ("yq2", [C, LQ], F32),
        ("yq3", [C, LQ], F32),
        ("x_res", [C, LQ], F32R), ("nin_wT", [C, C], F32R),
        ("s1", [C, 1], F32), ("t1", [C, 1], F32),
        ("nin2_wT", [C, C], F32R), ("s2", [C, 1], F32), ("t2", [C, 1], F32),
    ]:
        din[name] = nc.dram_tensor(name, shape, dt, kind="ExternalInput").ap()
    o_out = nc.dram_tensor("out_q", [C, LQ], F32, kind="ExternalOutput").ap()

    LC2 = 512
    from contextlib import ExitStack
    with tile.TileContext(nc) as tc, ExitStack() as es:
        pool = es.enter_context(tc.tile_pool(name="p2", bufs=1))
        ch2 = es.enter_context(tc.tile_pool(name="ch2", bufs=3))
        psum = es.enter_context(tc.tile_pool(name="ps2", bufs=4, space="PSUM"))
        t = {}
        for name in ("nin_wT", "s1", "t1", "nin2_wT", "s2", "t2"):
            t[name] = pool.tile(list(din[name].shape),
                                din[name].tensor.dtype, name=f"t_{name}")
            nc.sync.dma_start(t[name], din[name])
        out_sb = pool.tile([C, LQ], F32, name="out_sb")
        for ci in range(LQ // LC2):
            sl = slice(ci * LC2, (ci + 1) * LC2)
            xs = ch2.tile([C, LC2], F32R, name="xs", tag="xs")
            nc.sync.dma_start(xs, din["x_res"][:, sl])
            yq = ch2.tile([C, 4, LC2], F32, name="yq", tag="yq")
            for q in range(4):
                nc.sync.dma_start(yq[:, q, :], din[f"yq{q}"][:, sl])
            ps = psum.tile([C, LC2], F32, name="ps_a", tag="ps2")
            nc.tensor.matmul(ps, t["nin_wT"], xs, start=True, stop=True)
            actq = ch2.tile([C, LC2], F32, name="actq", tag="actq")
            nc.scalar.activation(actq, ps, AF.Relu,
                                 scale=t["s1"][:, 0:1], bias=t["t1"][:, 0:1])
            a0 = ch2.tile([C, LC2], F32, name="a0", tag="a0")
            nc.vector.tensor_add(a0, yq[:, 0, :], yq[:, 1, :])
            a1 = ch2.tile([C, LC2], F32, name="a1", tag="a1")
            nc.vector.tensor_add(a1, yq[:, 2, :], yq[:, 3, :])
            a2 = ch2.tile([C, LC2], F32, name="a2", tag="a2")
            nc.vector.tensor_add(a2, a0, a1)
            pre = ch2.tile([C, LC2], F32R, name="pre", tag="pre")
            nc.vector.tensor_add(pre, a2, actq)
            ps2 = psum.tile([C, LC2], F32, name="ps_b", tag="ps2")
            nc.tensor.matmul(ps2, t["nin2_wT"], pre, start=True, stop=True)
            nc.scalar.activation(out_sb[:, sl], ps2, AF.Relu,
                                 scale=t["s2"][:, 0:1], bias=t["t2"][:, 0:1])
        nc.sync.dma_start(o_out, out_sb)
    nc.compile()
    return nc


# ---------------------------------------------------------------- runner
class _Cached:
    def __init__(self, nc):
        bass2jax.install_neuronx_cc_hook()
        self.nc = nc
        in_names, out_names, out_avals, zero_shapes = [], [], [], []
        pname = nc.partition_id_tensor.name if nc.partition_id_tensor else None
        for alloc in nc.m.functions[0].allocations:
            if not isinstance(alloc, mybir.MemoryLocationSet):
                continue
            name = alloc.memorylocations[0].name
            if alloc.kind == "ExternalInput":
                if name != pname:
                    in_names.append(name)
            elif alloc.kind == "ExternalOutput":
                out_names.append(name)
                shape = tuple(alloc.tensor_shape)
                dtype = mybir.dt.np(alloc.dtype)
                out_avals.append(jax.core.ShapedArray(shape, dtype))
                zero_shapes.append((shape, dtype))
        self.in_names, self.out_names = in_names, out_names
        self.out_avals, self.zero_shapes = out_avals, zero_shapes
        n_params, n_outs = len(in_names), len(out_names)
        all_in = list(in_names) + list(out_names)
        if pname is not None:
            all_in.append(pname)

        def _body(*args):
            operands = list(args)
            if pname is not None:
                operands.append(bass2jax.partition_id_tensor())
            return tuple(bass2jax._bass_exec_p.bind(
                *operands, out_avals=tuple(out_avals), in_names=tuple(all_in),
                out_names=tuple(out_names), lowering_input_output_aliases=(),
                sim_require_finite=True, sim_require_nnan=True, nc=nc))

        devices = jax.devices()[:N_CORES]
        mesh = Mesh(np.asarray(devices), ("core",))
        self.sharded = jax.jit(
            shard_map(_body, mesh=mesh,
                      in_specs=(PartitionSpec("core"),) * (n_params + n_outs),
                      out_specs=(PartitionSpec("core"),) * n_outs,
                      check_rep=False),
            donate_argnums=tuple(range(n_params, n_params + n_outs)),
            keep_unused=True)

    def run(self, in_maps):
        cc = [np.concatenate([np.ascontiguousarray(
                np.asarray(in_maps[c][nm], dtype=self._np_dtype(nm)))
              for c in range(N_CORES)], axis=0) for nm in self.in_names]
        zz = [np.zeros((N_CORES * s[0], *s[1:]), d)
              for (s, d) in self.zero_shapes]
        out = self.sharded(*cc, *zz)
        return [
            {nm: np.asarray(out[i]).reshape(N_CORES, *self.out_avals[i].shape)[c]
             for i, nm in enumerate(self.out_names)}
            for c in range(N_CORES)
        ]

    def _np_dtype(self, nm):
        for alloc in self.nc.m.functions[0].allocations:
            if (isinstance(alloc, mybir.MemoryLocationSet)
                    and alloc.memorylocations[0].name == nm):
                return mybir.dt.np(alloc.dtype)
        return np.float32


_CACHE = {}


def _get(key, builder):
    if key not in _CACHE:
        with _pinned_act_set():
            _CACHE[key] = _Cached(builder())
    return _CACHE[key]


# ---------------------------------------------------------------- host glue
def _selectors():
    """(n,ds) layout helpers.  partition index = n*DS + ds; d = g*DS + ds."""
    sel_b = np.zeros((DTR + 2 * N, DIN), np.float32)
    sel_c = np.zeros((DTR + 2 * N, DIN), np.float32)
    selg = np.zeros((DIN, NG * DIN), np.float32)
    onesg = np.zeros((DIN, NG * DIN), np.float32)
    for n in range(N):
        for ds in range(DS):
            prt = n * DS + ds
            sel_b[DTR + n, prt] = 1.0
            sel_c[DTR + N + n, prt] = 1.0
            for g in range(NG):
                d = g * DS + ds
                selg[d, g * DIN + prt] = 1.0
                onesg[prt, g * DIN + d] = 1.0
    return sel_b, sel_c, selg, onesg


def kernel(**inputs):
    x = np.asarray(inputs["x"], np.float32).reshape(B, C, L)
    s1 = (np.asarray(inputs["g1"]) / np.sqrt(np.asarray(inputs["v1"]) + BN_EPS)
          ).astype(np.float32)
    t1 = (np.asarray(inputs["b1"]) - np.asarray(inputs["m1"]) * s1
          ).astype(np.float32)
    s2 = (np.asarray(inputs["g2"]) / np.sqrt(np.asarray(inputs["v2"]) + BN_EPS)
          ).astype(np.float32)
    t2 = (np.asarray(inputs["b2"]) - np.asarray(inputs["m2"]) * s2
          ).astype(np.float32)
    w_in = np.asarray(inputs["w_in"], np.float32)
    w_out = np.asarray(inputs["w_out"], np.float32)
    conv_w = np.asarray(inputs["conv_w"], np.float32)
    conv_b = np.asarray(inputs["conv_b"], np.float32)
    A_neg = (-np.exp(np.asarray(inputs["A_log"]))).astype(np.float32)
    D_param = np.asarray(inputs["D_param"], np.float32)
    nin_wT = np.ascontiguousarray(np.asarray(inputs["nin_w"], np.float32).T)
    nin2_wT = np.ascontiguousarray(np.asarray(inputs["nin2_w"], np.float32).T)
    conv_diag = np.zeros((DIN, DCONV * DIN), np.float32)
    for k in range(DCONV):
        conv_diag[:, k * DIN:(k + 1) * DIN][np.arange(DIN), np.arange(DIN)] = \
            conv_w[:, k]
    # A_perm[(n,ds), g] = A_neg[g*DS+ds, n]
    A_perm = np.zeros((DIN, NG), np.float32)
    for n in range(N):
        for ds in range(DS):
            A_perm[n * DS + ds, :] = A_neg[np.arange(NG) * DS + ds, n]
    sel_b, sel_c, selg, onesg = _selectors()

    k1 = _get("n1", _build_neff1)
    k2 = _get("n2", _build_neff2)

    com = dict(
        nin_wT=nin_wT, s1=s1[:, None], t1=t1[:, None],
        conv_diag=conv_diag, conv_b=conv_b[:, None],
        conv_bn=(-conv_b)[:, None],
        w_xproj=np.asarray(inputs["w_xproj"], np.float32),
        w_dt=np.asarray(inputs["w_dt"], np.float32),
        b_dt=np.asarray(inputs["b_dt"], np.float32)[:, None],
        A_perm=A_perm, D_diag=np.diag(D_param).astype(np.float32),
        SEL_B=sel_b, SEL_C=sel_c, SELG=selg, ONESG=onesg.astype(ml_bf16),
    )
    in1 = []
    for core in range(N_CORES):
        b, d = core // 4, core % 4
        cflip, lflip = d in (1, 3), d in (2, 3)
        wi = w_in[::-1].copy() if cflip else w_in
        wo = (w_out[:, ::-1].copy() if cflip else w_out) / 4.0
        xb = x[b][:, ::-1].copy() if lflip else x[b]
        m = dict(com)
        m.update(xb=xb, w_in_x=np.ascontiguousarray(wi[:, :DIN]),
                 w_in_z=np.ascontiguousarray(wi[:, DIN:]),
                 w_out_q=np.ascontiguousarray(wo))
        in1.append(m)
    res1 = k1.run(in1)

    ys = []
    for core in range(N_CORES):
        y = res1[core]["y_dir"]
        if core % 4 in (2, 3):
            y = y[:, ::-1]
        ys.append(y)

    in2 = []
    for core in range(N_CORES):
        b, q = core // 4, core % 4
        sl = slice(q * LQ, (q + 1) * LQ)
        m = dict(
            yq0=np.ascontiguousarray(ys[b * 4 + 0][:, sl]),
            yq1=np.ascontiguousarray(ys[b * 4 + 1][:, sl]),
            yq2=np.ascontiguousarray(ys[b * 4 + 2][:, sl]),
            yq3=np.ascontiguousarray(ys[b * 4 + 3][:, sl]),
            x_res=np.ascontiguousarray(x[b][:, sl]),
            nin_wT=nin_wT, s1=s1[:, None], t1=t1[:, None],
            nin2_wT=nin2_wT, s2=s2[:, None], t2=t2[:, None],
        )
        in2.append(m)
    res2 = k2.run(in2)

    out = np.zeros((B, C, L), np.float32)
    for core in range(N_CORES):
        b, q = core // 4, core % 4
        out[b, :, q * LQ:(q + 1) * LQ] = res2[core]["out_q"]
    return out.reshape(B, C, DZ, H, W)


# revision 30
# speedup vs baseline: 1.0007x; 1.0007x over previous
"""Trainium2 Bass kernel for nn_CFCML_20083267076887 (4-direction Mamba-style
selective-scan block between two 1x1 conv+BN+ReLU stages).

Sharding: 8 cores = (batch b in {0,1}) x (scan direction d in {0..3}).
 - channel flips (dirs 1,3) fold into w_in rows / w_out cols on host
 - L flips (dirs 2,3) feed the core a host-flipped x slice; host unflips the
   core's y output before the combine stage (pure data movement)

NEFF1 (per core): the selective scan runs with SBUF partitions holding
(state n, channel-sub ds) pairs: partition = n*8 + ds, over 16 channel
groups g (d = g*8 + ds), chunked at LC=1024 along L.
 - all GEMMs in fp32r (1 cyc/col on the PE vs 4 for fp32; ~1e-4 rel err)
 - one ACT table set (natural_log_exp_and_others) pinned at compile time;
   softplus(x) = Ln(Exp(x)+1) and silu(x) = x*Exp(-Ln(Exp(-x)+1)) exactly
 - B/C rows broadcast once per chunk into (n,ds) layout via 0/1 selector
   matmuls; delta/wdx replicated per group the same way (SELG)
 - per group: ACT computes dA = exp(A*delta_rep) from PSUM; ACT evacuates
   wdx_rep to bf16 so the dBu mul runs in the DVE 2x bf16 mode; DVE runs
   the tensor_tensor_scan (fp32 state, bf16 h out) and p = h*Cb (bf16 2x);
   PE accumulates y over groups in PSUM via 0/1 matmuls + D*xm as a
   diagonal matmul
 - emission is software-pipelined: chunk k+1's GEMM/ACT stages are emitted
   inside chunk k's group loop, and group g's y-acc matmul is emitted
   during group g+1, so DVE/PE/ACT queues stay dense across chunk
   boundaries (DVE ~99% busy)

NEFF2 (per core = (b, quarter)): slab-pipelined sum of 4 direction y
quarters + recomputed conv1 residual -> 1x1 conv2 + BN + ReLU.
"""
import sys
import numpy as np

for _p in ("/opt/trn_rl_repo", "/root/.axon_site/_ro/trn_rl_repo"):
    if _p not in sys.path:
        sys.path.append(_p)

import jax
from jax.sharding import Mesh, PartitionSpec
from jax.experimental.shard_map import shard_map

import concourse.bacc as bacc
import concourse.tile as tile
import concourse.mybir as mybir
from concourse import bass2jax

# Pin every ACTIVATE to the natural_log_exp_and_others table set (it holds
# exp/ln/relu/copy — everything these NEFFs use).  The stock per-function
# greedy choice alternates exp_and_others <-> natural_log on every Exp<->Ln
# boundary, inserting ~73 ACT_TABLE_LOADs (~94us) per NEFF1 run.  Emptying
# the other sets (ids preserved) makes the fixpoint hoist a single load.
import contextlib
import concourse.hw_specs as _hw_specs
_PIN_SET = "natural_log_exp_and_others"


@contextlib.contextmanager
def _pinned_act_set():
    orig = _hw_specs.get_activation_tables

    def _pinned(module_arch):
        tabs = orig(module_arch)
        return {name: (fns if name == _PIN_SET else set())
                for name, fns in tabs.items()}

    _hw_specs.get_activation_tables = _pinned
    bacc.get_activation_tables = _pinned
    try:
        yield
    finally:
        _hw_specs.get_activation_tables = orig
        bacc.get_activation_tables = orig

F32 = mybir.dt.float32
F32R = mybir.dt.float32r
BF16 = mybir.dt.bfloat16
AF = mybir.ActivationFunctionType
OP = mybir.AluOpType
ml_bf16 = mybir.dt.np(BF16)

B, C, DZ, H, W = 2, 64, 12, 32, 32
N = 16
DCONV = 4
DIN = 128
DTR = 4
L = DZ * H * W          # 12288
LQ = L // 4             # 3072
BN_EPS = 1e-5
LC = 1024
NCH = L // LC
NG = 16                 # channel groups of 8
DS = DIN // NG          # 8
N_CORES = 8


# ---------------------------------------------------------------- NEFF 1
def _build_neff1():
    nc = bacc.Bacc("TRN2", target_bir_lowering=False, debug=False,
                   num_devices=N_CORES)
    din = {}
    for name, shape, dt in [
        ("xb", [C, L], F32R),
        ("nin_wT", [C, C], F32R), ("s1", [C, 1], F32), ("t1", [C, 1], F32),
        ("w_in_x", [C, DIN], F32R), ("w_in_z", [C, DIN], F32R),
        ("conv_diag", [DIN, DCONV * DIN], F32R), ("conv_b", [DIN, 1], F32),
        ("conv_bn", [DIN, 1], F32),
        ("w_xproj", [DIN, DTR + 2 * N], F32R),
        ("w_dt", [DTR, DIN], F32R), ("b_dt", [DIN, 1], F32),
        ("A_perm", [DIN, NG], F32), ("D_diag", [DIN, DIN], F32R),
        ("w_out_q", [DIN, C], F32R),
        ("SEL_B", [DTR + 2 * N, DIN], F32R), ("SEL_C", [DTR + 2 * N, DIN], F32R),
        ("SELG", [DIN, NG * DIN], F32R), ("ONESG", [DIN, NG * DIN], BF16),
    ]:
        din[name] = nc.dram_tensor(name, shape, dt, kind="ExternalInput").ap()
    y_out = nc.dram_tensor("y_dir", [C, L], F32, kind="ExternalOutput").ap()

    def mm(ps, lhsT, rhs, start=True, stop=True):
        fd = rhs.shape[-1]
        for s0 in range(0, fd, 512):
            s1 = min(s0 + 512, fd)
            nc.tensor.matmul(ps[:, s0:s1], lhsT, rhs[:, s0:s1],
                             start=start, stop=stop)

    def mm_acc(ps, parts):
        fd = parts[0][1].shape[-1]
        for s0 in range(0, fd, 512):
            s1 = min(s0 + 512, fd)
            for k, (lhsT, rhs) in enumerate(parts):
                nc.tensor.matmul(ps[:, s0:s1], lhsT, rhs[:, s0:s1],
                                 start=(k == 0), stop=(k == len(parts) - 1))

    SL = 512
    NSL = LC // SL

    from contextlib import ExitStack
    with tile.TileContext(nc) as tc, ExitStack() as es:
        wp = es.enter_context(tc.tile_pool(name="wp", bufs=1))
        sp = es.enter_context(tc.tile_pool(name="sp", bufs=2))
        gp = es.enter_context(tc.tile_pool(name="gp", bufs=3))
        tp = es.enter_context(tc.tile_pool(name="tp", bufs=4))
        pp = es.enter_context(tc.tile_pool(name="pp", bufs=1))
        ap = es.enter_context(tc.tile_pool(name="ap", bufs=4))
        psG = es.enter_context(tc.tile_pool(name="psG", bufs=2, space="PSUM"))
        psY = es.enter_context(tc.tile_pool(name="psY", bufs=1, space="PSUM"))
        psR = es.enter_context(tc.tile_pool(name="psR", bufs=2, space="PSUM"))

        w = {}
        for name in din:
            if name == "xb":
                continue
            t = wp.tile(list(din[name].shape), din[name].tensor.dtype,
                        name=f"w_{name}")
            nc.sync.dma_start(t, din[name])
            w[name] = t
        carry = wp.tile([DIN, NG], F32, name="carry")

        # ~5us of dense matmuls at NEFF start: ramps the PE HAM clock-gate
        # to 8/8 so the steady-state mms run at 2.4 GHz from the first chunk
        pswu = psG.tile([DIN, 512], F32, name="pswu", tag="psG")
        for _ in range(12):
            nc.tensor.matmul(pswu, w["SELG"][:, 0:DIN],
                             w["SELG"][:, 0:512], start=True, stop=True)

        # ---------------- pipelined emission ----------------
        # emit_gemm_stages(ch) returns 4 closures (S0..S3) that produce the
        # chunk's GEMM-phase tensors; they are interleaved into the PREVIOUS
        # chunk's group loop so PE/ACT/DVE queues stay dense across chunks.
        states = {}

        def emit_gemm_stages(ch):
            st = {}
            states[ch] = st
            lo = ch * LC

            def S0():
                x_t = sp.tile([C, LC], F32R, name="x_t", tag="x_t")
                nc.sync.dma_start(x_t, din["xb"][:, lo:lo + LC])
                act = sp.tile([C, LC], F32R, name="act", tag="act")
                for s in range(NSL):
                    sl = slice(s * SL, (s + 1) * SL)
                    ps = psG.tile([C, SL], F32, name="ps_h0", tag="psG")
                    nc.tensor.matmul(ps, w["nin_wT"], x_t[:, sl],
                                     start=True, stop=True)
                    nc.scalar.activation(act[:, sl], ps, AF.Relu,
                                         scale=w["s1"][:, 0:1],
                                         bias=w["t1"][:, 0:1])
                xmpre = sp.tile([DIN, LC + 3], F32R, name="xmpre", tag="xmpre")
                for s in range(NSL):
                    sl = slice(s * SL, (s + 1) * SL)
                    ps = psG.tile([DIN, SL], F32, name="ps_xx", tag="psG")
                    nc.tensor.matmul(ps, w["w_in_x"], act[:, sl],
                                     start=True, stop=True)
                    nc.scalar.copy(xmpre[:, 3 + s * SL:3 + (s + 1) * SL], ps)
                if ch == 0:
                    nc.vector.memset(xmpre[:, 0:3].bitcast(F32), 0.0)
                else:
                    prev = states[ch - 1]["xmpre"]
                    nc.scalar.copy(xmpre[:, 0:3], prev[:, LC:LC + 3])
                st["act"], st["xmpre"] = act, xmpre

            def S1():
                act, xmpre = st["act"], st["xmpre"]
                xm = sp.tile([DIN, LC], F32R, name="xm", tag="xm")
                for s in range(NSL):
                    sl = slice(s * SL, (s + 1) * SL)
                    psc = psG.tile([DIN, SL], F32, name="ps_xc", tag="psG")
                    for k in range(DCONV):
                        nc.tensor.matmul(
                            psc, w["conv_diag"][:, k * DIN:(k + 1) * DIN],
                            xmpre[:, k + s * SL:k + s * SL + SL],
                            start=(k == 0), stop=(k == DCONV - 1))
                    s1_ = tp.tile([DIN, SL], F32, name="s1c", tag="tmp")
                    nc.scalar.activation(s1_, psc, AF.Exp, scale=-1.0,
                                         bias=w["conv_bn"][:, 0:1])
                    s2_ = tp.tile([DIN, SL], F32, name="s2c", tag="tmp")
                    nc.scalar.activation(s2_, s1_, AF.Ln, bias=1.0)
                    sg_ = tp.tile([DIN, SL], F32, name="sgc", tag="tmp")
                    nc.scalar.activation(sg_, s2_, AF.Exp, scale=-1.0)
                    nc.vector.scalar_tensor_tensor(
                        xm[:, sl], psc, w["conv_b"][:, 0:1], sg_,
                        OP.add, OP.mult)
                st["xm"] = xm

            def S2():
                xm = st["xm"]
                NBC = DTR + 2 * N
                bcS = sp.tile([NBC, LC], F32R, name="bcS", tag="bcS")
                dwx = sp.tile([DIN, 2 * LC], F32R, name="dwx", tag="dwx")
                delta = dwx[:, 0:LC]
                for s in range(NSL):
                    sl = slice(s * SL, (s + 1) * SL)
                    psbc = psG.tile([NBC, SL], F32, name="ps_bc", tag="psG")
                    nc.tensor.matmul(psbc, w["w_xproj"], xm[:, sl],
                                     start=True, stop=True)
                    nc.scalar.copy(bcS[:, sl], psbc)
                for s in range(NSL):
                    sl = slice(s * SL, (s + 1) * SL)
                    psp = psG.tile([DIN, SL], F32, name="ps_dpre", tag="psG")
                    nc.tensor.matmul(psp, w["w_dt"], bcS[0:DTR, sl],
                                     start=True, stop=True)
                    e_ = tp.tile([DIN, SL], F32, name="e_", tag="tmp")
                    nc.scalar.activation(e_, psp, AF.Exp,
                                         bias=w["b_dt"][:, 0:1])
                    nc.scalar.activation(delta[:, sl], e_, AF.Ln, bias=1.0)
                nc.vector.tensor_tensor(dwx[:, LC:2 * LC], delta, xm, OP.mult)
                st["dwx"], st["bcS"] = dwx, bcS

            def S3():
                act = st["act"]
                zs = sp.tile([DIN, LC], F32, name="zs", tag="zs")
                for s in range(NSL):
                    sl = slice(s * SL, (s + 1) * SL)
                    psz = psG.tile([DIN, SL], F32, name="ps_z", tag="psG")
                    nc.tensor.matmul(psz, w["w_in_z"], act[:, sl],
                                     start=True, stop=True)
                    z1 = tp.tile([DIN, SL], F32, name="z1", tag="tmp")
                    nc.scalar.activation(z1, psz, AF.Exp, scale=-1.0)
                    z2 = tp.tile([DIN, SL], F32, name="z2", tag="tmp")
                    nc.scalar.activation(z2, z1, AF.Ln, bias=1.0)
                    z3 = tp.tile([DIN, SL], F32, name="z3", tag="tmp")
                    nc.scalar.activation(z3, z2, AF.Exp, scale=-1.0)
                    nc.vector.tensor_tensor(zs[:, sl], psz, z3, OP.mult)
                st["zs"] = zs

            return [S0, S1, S2, S3]

        def emit_bbcb(st):
            bcS = st["bcS"]
            psB = psR.tile([DIN, LC], F32, name="psB", tag="psR")
            mm(psB, w["SEL_B"], bcS)
            Bb = sp.tile([DIN, LC], BF16, name="Bb", tag="Bb")
            nc.scalar.copy(Bb, psB)
            psC = psR.tile([DIN, LC], F32, name="psC", tag="psR")
            mm(psC, w["SEL_C"], bcS)
            Cb = sp.tile([DIN, LC], BF16, name="Cb", tag="Cb")
            nc.scalar.copy(Cb, psC)
            st["Bb"], st["Cb"] = Bb, Cb

        # prologue: chunk 0 GEMM phase + Bb/Cb
        for f in emit_gemm_stages(0):
            f()
        emit_bbcb(states[0])

        for ch in range(NCH):
            st = states[ch]
            delta = st["dwx"][:, 0:LC]
            wdx = st["dwx"][:, LC:2 * LC]
            nxt = emit_gemm_stages(ch + 1) if ch + 1 < NCH else []
            psy = psY.tile([DIN, LC], F32, name="psy", tag="psy")
            p_tiles = {}
            for g in range(NG):
                sel_g = w["SELG"][:, g * DIN:(g + 1) * DIN]
                psdD = psR.tile([DIN, LC], F32, name="psdD", tag="psR")
                mm(psdD, sel_g, delta)
                dA = ap.tile([DIN, LC], F32, name="dA", tag="dA")
                nc.scalar.activation(dA, psdD, AF.Exp,
                                     scale=w["A_perm"][:, g:g + 1])
                psdW = psR.tile([DIN, LC], F32, name="psdW", tag="psR")
                mm(psdW, sel_g, wdx)
                wdxR = ap.tile([DIN, LC], BF16, name="wdxR", tag="wdxR")
                nc.scalar.copy(wdxR, psdW)
                dBu = ap.tile([DIN, LC], BF16, name="dBu", tag="dBu")
                nc.vector.tensor_tensor(dBu, wdxR, st["Bb"], OP.mult)
                h = gp.tile([DIN, LC], BF16, name="h", tag="h")
                init = 0.0 if ch == 0 else carry[:, g:g + 1]
                nc.vector.tensor_tensor_scan(h, dA, dBu, init,
                                             OP.mult, OP.add)
                nc.scalar.copy(carry[:, g:g + 1], h[:, LC - 1:LC])
                p = gp.tile([DIN, LC], BF16, name="p", tag="p")
                nc.vector.tensor_tensor(p, h, st["Cb"], OP.mult)
                p_tiles[g] = p
                # y-acc for the previous group (keeps PE stream dense)
                if g >= 1:
                    mm(psy, w["ONESG"][:, (g - 1) * DIN:g * DIN],
                       p_tiles.pop(g - 1), start=(g == 1), stop=False)
                # interleave next chunk's GEMM stages
                if nxt:
                    if g == 4:
                        nxt[0]()
                    elif g == 7:
                        nxt[1]()
                    elif g == 10:
                        nxt[2]()
                    elif g == 13:
                        nxt[3]()
            mm(psy, w["ONESG"][:, (NG - 1) * DIN:NG * DIN],
               p_tiles.pop(NG - 1), start=False, stop=False)
            # + D * xm (diagonal matmul closes the accumulation group)
            mm(psy, w["D_diag"], st["xm"], start=False, stop=True)

            # y = psy * zs ; project by w_out
            y = sp.tile([DIN, LC], F32R, name="y", tag="y")
            nc.vector.tensor_tensor(y, psy, st["zs"], OP.mult)
            yo = sp.tile([C, LC], F32, name="yo", tag="yo")
            for s in range(NSL):
                sl = slice(s * SL, (s + 1) * SL)
                pso = psG.tile([C, SL], F32, name="ps_yo", tag="psG")
                nc.tensor.matmul(pso, w["w_out_q"], y[:, sl],
                                 start=True, stop=True)
                nc.scalar.copy(yo[:, sl], pso)
            nc.sync.dma_start(y_out[:, ch * LC:(ch + 1) * LC], yo)
            if nxt:
                emit_bbcb(states[ch + 1])

    nc.compile()
    return nc


# ---------------------------------------------------------------- NEFF 2
def _build_neff2():
    nc = bacc.Bacc("TRN2", target_bir_lowering=False, debug=False,
                   num_devices=N_CORES)
    din = {}
    for name, shape, dt in [
        ("yq0", [C, LQ], F32), ("yq1", [C, LQ], F32), ("yq2", [C, LQ], F32),
        ("yq3", [C, LQ], F32),
        ("x_res", [C, LQ], F32R), ("nin_wT", [C, C], F32R),
        ("s1", [C, 1], F32), ("t1", [C, 1], F32),
        ("nin2_wT", [C, C], F32R), ("s2", [C, 1], F32), ("t2", [C, 1], F32),
    ]:
        din[name] = nc.dram_tensor(name, shape, dt, kind="ExternalInput").ap()
    o_out = nc.dram_tensor("out_q", [C, LQ], F32, kind="ExternalOutput").ap()

    LC2 = 512
    from contextlib import ExitStack
    with tile.TileContext(nc) as tc, ExitStack() as es:
        pool = es.enter_context(tc.tile_pool(name="p2", bufs=1))
        ch2 = es.enter_context(tc.tile_pool(name="ch2", bufs=3))
        psum = es.enter_context(tc.tile_pool(name="ps2", bufs=4, space="PSUM"))
        t = {}
        for name in ("nin_wT", "s1", "t1", "nin2_wT", "s2", "t2"):
            t[name] = pool.tile(list(din[name].shape),
                                din[name].tensor.dtype, name=f"t_{name}")
            nc.sync.dma_start(t[name], din[name])
        out_sb = pool.tile([C, LQ], F32, name="out_sb")
        for ci in range(LQ // LC2):
            sl = slice(ci * LC2, (ci + 1) * LC2)
            xs = ch2.tile([C, LC2], F32R, name="xs", tag="xs")
            nc.sync.dma_start(xs, din["x_res"][:, sl])
            yq = ch2.tile([C, 4, LC2], F32, name="yq", tag="yq")
            for q in range(4):
                nc.sync.dma_start(yq[:, q, :], din[f"yq{q}"][:, sl])
            ps = psum.tile([C, LC2], F32, name="ps_a", tag="ps2")
            nc.tensor.matmul(ps, t["nin_wT"], xs, start=True, stop=True)
            actq = ch2.tile([C, LC2], F32, name="actq", tag="actq")
            nc.scalar.activation(actq, ps, AF.Relu,
                                 scale=t["s1"][:, 0:1], bias=t["t1"][:, 0:1])
            a0 = ch2.tile([C, LC2], F32, name="a0", tag="a0")
            nc.vector.tensor_add(a0, yq[:, 0, :], yq[:, 1, :])
            a1 = ch2.tile([C, LC2], F32, name="a1", tag="a1")
            nc.vector.tensor_add(a1, yq[:, 2, :], yq[:, 3, :])
            a2 = ch2.tile([C, LC2], F32, name="a2", tag="a2")
            nc.vector.tensor_add(a2, a0, a1)
            pre = ch2.tile([C, LC2], F32R, name="pre", tag="pre")
            nc.vector.tensor_add(pre, a2, actq)
            ps2 = psum.tile([C, LC2], F32, name="ps_b", tag="ps2")
            nc.tensor.matmul(ps2, t["nin2_wT"], pre, start=True, stop=True)
            nc.scalar.activation(out_sb[:, sl], ps2, AF.Relu,
                                 scale=t["s2"][:, 0:1], bias=t["t2"][:, 0:1])
        nc.sync.dma_start(o_out, out_sb)
    nc.compile()
    return nc


# ---------------------------------------------------------------- runner
class _Cached:
    def __init__(self, nc):
        bass2jax.install_neuronx_cc_hook()
        self.nc = nc
        in_names, out_names, out_avals, zero_shapes = [], [], [], []
        pname = nc.partition_id_tensor.name if nc.partition_id_tensor else None
        for alloc in nc.m.functions[0].allocations:
            if not isinstance(alloc, mybir.MemoryLocationSet):
                continue
            name = alloc.memorylocations[0].name
            if alloc.kind == "ExternalInput":
                if name != pname:
                    in_names.append(name)
            elif alloc.kind == "ExternalOutput":
                out_names.append(name)
                shape = tuple(alloc.tensor_shape)
                dtype = mybir.dt.np(alloc.dtype)
                out_avals.append(jax.core.ShapedArray(shape, dtype))
                zero_shapes.append((shape, dtype))
        self.in_names, self.out_names = in_names, out_names
        self.out_avals, self.zero_shapes = out_avals, zero_shapes
        n_params, n_outs = len(in_names), len(out_names)
        all_in = list(in_names) + list(out_names)
        if pname is not None:
            all_in.append(pname)

        def _body(*args):
            operands = list(args)
            if pname is not None:
                operands.append(bass2jax.partition_id_tensor())
            return tuple(bass2jax._bass_exec_p.bind(
                *operands, out_avals=tuple(out_avals), in_names=tuple(all_in),
                out_names=tuple(out_names), lowering_input_output_aliases=(),
                sim_require_finite=True, sim_require_nnan=True, nc=nc))

        devices = jax.devices()[:N_CORES]
        mesh = Mesh(np.asarray(devices), ("core",))
        self.sharded = jax.jit(
            shard_map(_body, mesh=mesh,
                      in_specs=(PartitionSpec("core"),) * (n_params + n_outs),
                      out_specs=(PartitionSpec("core"),) * n_outs,
                      check_rep=False),
            donate_argnums=tuple(range(n_params, n_params + n_outs)),
            keep_unused=True)

    def run(self, in_maps):
        cc = [np.concatenate([np.ascontiguousarray(
                np.asarray(in_maps[c][nm], dtype=self._np_dtype(nm)))
              for c in range(N_CORES)], axis=0) for nm in self.in_names]
        zz = [np.zeros((N_CORES * s[0], *s[1:]), d)
              for (s, d) in self.zero_shapes]
        out = self.sharded(*cc, *zz)
        return [
            {nm: np.asarray(out[i]).reshape(N_CORES, *self.out_avals[i].shape)[c]
             for i, nm in enumerate(self.out_names)}
            for c in range(N_CORES)
        ]

    def _np_dtype(self, nm):
        for alloc in self.nc.m.functions[0].allocations:
            if (isinstance(alloc, mybir.MemoryLocationSet)
                    and alloc.memorylocations[0].name == nm):
                return mybir.dt.np(alloc.dtype)
        return np.float32


_CACHE = {}


def _get(key, builder):
    if key not in _CACHE:
        with _pinned_act_set():
            _CACHE[key] = _Cached(builder())
    return _CACHE[key]


# ---------------------------------------------------------------- host glue
def _selectors():
    """(n,ds) layout helpers.  partition index = n*DS + ds; d = g*DS + ds."""
    sel_b = np.zeros((DTR + 2 * N, DIN), np.float32)
    sel_c = np.zeros((DTR + 2 * N, DIN), np.float32)
    selg = np.zeros((DIN, NG * DIN), np.float32)
    onesg = np.zeros((DIN, NG * DIN), np.float32)
    for n in range(N):
        for ds in range(DS):
            prt = n * DS + ds
            sel_b[DTR + n, prt] = 1.0
            sel_c[DTR + N + n, prt] = 1.0
            for g in range(NG):
                d = g * DS + ds
                selg[d, g * DIN + prt] = 1.0
                onesg[prt, g * DIN + d] = 1.0
    return sel_b, sel_c, selg, onesg


def kernel(**inputs):
    x = np.asarray(inputs["x"], np.float32).reshape(B, C, L)
    s1 = (np.asarray(inputs["g1"]) / np.sqrt(np.asarray(inputs["v1"]) + BN_EPS)
          ).astype(np.float32)
    t1 = (np.asarray(inputs["b1"]) - np.asarray(inputs["m1"]) * s1
          ).astype(np.float32)
    s2 = (np.asarray(inputs["g2"]) / np.sqrt(np.asarray(inputs["v2"]) + BN_EPS)
          ).astype(np.float32)
    t2 = (np.asarray(inputs["b2"]) - np.asarray(inputs["m2"]) * s2
          ).astype(np.float32)
    w_in = np.asarray(inputs["w_in"], np.float32)
    w_out = np.asarray(inputs["w_out"], np.float32)
    conv_w = np.asarray(inputs["conv_w"], np.float32)
    conv_b = np.asarray(inputs["conv_b"], np.float32)
    A_neg = (-np.exp(np.asarray(inputs["A_log"]))).astype(np.float32)
    D_param = np.asarray(inputs["D_param"], np.float32)
    nin_wT = np.ascontiguousarray(np.asarray(inputs["nin_w"], np.float32).T)
    nin2_wT = np.ascontiguousarray(np.asarray(inputs["nin2_w"], np.float32).T)
    conv_diag = np.zeros((DIN, DCONV * DIN), np.float32)
    for k in range(DCONV):
        conv_diag[:, k * DIN:(k + 1) * DIN][np.arange(DIN), np.arange(DIN)] = \
            conv_w[:, k]
    # A_perm[(n,ds), g] = A_neg[g*DS+ds, n]
    A_perm = np.zeros((DIN, NG), np.float32)
    for n in range(N):
        for ds in range(DS):
            A_perm[n * DS + ds, :] = A_neg[np.arange(NG) * DS + ds, n]
    sel_b, sel_c, selg, onesg = _selectors()

    k1 = _get("n1", _build_neff1)
    k2 = _get("n2", _build_neff2)

    com = dict(
        nin_wT=nin_wT, s1=s1[:, None], t1=t1[:, None],
        conv_diag=conv_diag, conv_b=conv_b[:, None],
        conv_bn=(-conv_b)[:, None],
        w_xproj=np.asarray(inputs["w_xproj"], np.float32),
        w_dt=np.asarray(inputs["w_dt"], np.float32),
        b_dt=np.asarray(inputs["b_dt"], np.float32)[:, None],
        A_perm=A_perm, D_diag=np.diag(D_param).astype(np.float32),
        SEL_B=sel_b, SEL_C=sel_c, SELG=selg, ONESG=onesg.astype(ml_bf16),
    )
    in1 = []
    for core in range(N_CORES):
        b, d = core // 4, core % 4
        cflip, lflip = d in (1, 3), d in (2, 3)
        wi = w_in[::-1].copy() if cflip else w_in
        wo = (w_out[:, ::-1].copy() if cflip else w_out) / 4.0
        xb = x[b][:, ::-1].copy() if lflip else x[b]
        m = dict(com)
        m.update(xb=xb, w_in_x=np.ascontiguousarray(wi[:, :DIN]),
                 w_in_z=np.ascontiguousarray(wi[:, DIN:]),
                 w_out_q=np.ascontiguousarray(wo))
        in1.append(m)
    res1 = k1.run(in1)

    ys = []
    for core in range(N_CORES):
        y = res1[core]["y_dir"]
        if core % 4 in (2, 3):
            y = y[:, ::-1]
        ys.append(y)

    in2 = []
    for core in range(N_CORES):
        b, q = core // 4, core % 4
        sl = slice(q * LQ, (q + 1) * LQ)
        m = dict(
            yq0=np.ascontiguousarray(ys[b * 4 + 0][:, sl]),
            yq1=np.ascontiguousarray(ys[b * 4 + 1][:, sl]),
            yq2=np.ascontiguousarray(ys[b * 4 + 2][:, sl]),
            yq3=np.ascontiguousarray(ys[b * 4 + 3][:, sl]),
            x_res=np.ascontiguousarray(x[b][:, sl]),
            nin_wT=nin_wT, s1=s1[:, None], t1=t1[:, None],
            nin2_wT=nin2_wT, s2=s2[:, None], t2=t2[:, None],
        )
        in2.append(m)
    res2 = k2.run(in2)

    out = np.zeros((B, C, L), np.float32)
    for core in range(N_CORES):
        b, q = core // 4, core % 4
        out[b, :, q * LQ:(q + 1) * LQ] = res2[core]["out_q"]
    return out.reshape(B, C, DZ, H, W)


# revision 31
# speedup vs baseline: 1.0136x; 1.0129x over previous
"""Trainium2 Bass kernel for nn_CFCML_20083267076887 (4-direction Mamba-style
selective-scan block between two 1x1 conv+BN+ReLU stages).

Sharding: 8 cores = (batch b in {0,1}) x (scan direction d in {0..3}).
 - channel flips (dirs 1,3) fold into w_in rows / w_out cols on host
 - L flips (dirs 2,3) feed the core a host-flipped x slice; host unflips the
   core's y output before the combine stage (pure data movement)

NEFF1 (per core): the selective scan runs with SBUF partitions holding
(state n, channel-sub ds) pairs: partition = n*8 + ds, over 16 channel
groups g (d = g*8 + ds), chunked at LC=1024 along L.
 - all GEMMs in fp32r (1 cyc/col on the PE vs 4 for fp32; ~1e-4 rel err)
 - one ACT table set (natural_log_exp_and_others) pinned at compile time;
   softplus(x) = Ln(Exp(x)+1) and silu(x) = x*Exp(-Ln(Exp(-x)+1)) exactly
 - B/C rows broadcast once per chunk into (n,ds) layout via 0/1 selector
   matmuls; delta/wdx replicated per group the same way (SELG)
 - per group: ACT computes dA = exp(A*delta_rep) from PSUM; ACT evacuates
   wdx_rep to bf16 so the dBu mul runs in the DVE 2x bf16 mode; DVE runs
   the tensor_tensor_scan (fp32 state, bf16 h out) and p = h*Cb (bf16 2x);
   PE accumulates y over groups in PSUM via 0/1 matmuls + D*xm as a
   diagonal matmul
 - emission is software-pipelined: chunk k+1's GEMM/ACT stages are emitted
   inside chunk k's group loop, and group g's y-acc matmul is emitted
   during group g+1, so DVE/PE/ACT queues stay dense across chunk
   boundaries (DVE ~99% busy)

NEFF2 (per core = (b, quarter)): slab-pipelined sum of 4 direction y
quarters + recomputed conv1 residual -> 1x1 conv2 + BN + ReLU.
"""
import sys
import numpy as np

for _p in ("/opt/trn_rl_repo", "/root/.axon_site/_ro/trn_rl_repo"):
    if _p not in sys.path:
        sys.path.append(_p)

import jax
from jax.sharding import Mesh, PartitionSpec
from jax.experimental.shard_map import shard_map

import concourse.bacc as bacc
import concourse.tile as tile
import concourse.mybir as mybir
from concourse import bass2jax

# Pin every ACTIVATE to the natural_log_exp_and_others table set (it holds
# exp/ln/relu/copy — everything these NEFFs use).  The stock per-function
# greedy choice alternates exp_and_others <-> natural_log on every Exp<->Ln
# boundary, inserting ~73 ACT_TABLE_LOADs (~94us) per NEFF1 run.  Emptying
# the other sets (ids preserved) makes the fixpoint hoist a single load.
import contextlib
import concourse.hw_specs as _hw_specs
_PIN_SET = "natural_log_exp_and_others"


@contextlib.contextmanager
def _pinned_act_set():
    orig = _hw_specs.get_activation_tables

    def _pinned(module_arch):
        tabs = orig(module_arch)
        return {name: (fns if name == _PIN_SET else set())
                for name, fns in tabs.items()}

    _hw_specs.get_activation_tables = _pinned
    bacc.get_activation_tables = _pinned
    try:
        yield
    finally:
        _hw_specs.get_activation_tables = orig
        bacc.get_activation_tables = orig

F32 = mybir.dt.float32
F32R = mybir.dt.float32r
BF16 = mybir.dt.bfloat16
AF = mybir.ActivationFunctionType
OP = mybir.AluOpType
ml_bf16 = mybir.dt.np(BF16)

B, C, DZ, H, W = 2, 64, 12, 32, 32
N = 16
DCONV = 4
DIN = 128
DTR = 4
L = DZ * H * W          # 12288
LQ = L // 4             # 3072
BN_EPS = 1e-5
LC = 1024
NCH = L // LC
NG = 16                 # channel groups of 8
DS = DIN // NG          # 8
N_CORES = 8


# ---------------------------------------------------------------- NEFF 1
def _build_neff1():
    nc = bacc.Bacc("TRN2", target_bir_lowering=False, debug=False,
                   num_devices=N_CORES)
    din = {}
    for name, shape, dt in [
        ("xb", [C, L], F32R),
        ("nin_wT", [C, C], F32R), ("s1", [C, 1], F32), ("t1", [C, 1], F32),
        ("w_in_x", [C, DIN], F32R), ("w_in_z", [C, DIN], F32R),
        ("conv_diag", [DIN, DCONV * DIN], F32R), ("conv_b", [DIN, 1], F32),
        ("conv_bn", [DIN, 1], F32),
        ("w_xproj", [DIN, DTR + 2 * N], F32R),
        ("w_dt", [DTR, DIN], F32R), ("b_dt", [DIN, 1], F32),
        ("A_perm", [DIN, NG], F32), ("D_diag", [DIN, DIN], F32R),
        ("w_out_q", [DIN, C], F32R),
        ("SEL_B", [DTR + 2 * N, DIN], F32R), ("SEL_C", [DTR + 2 * N, DIN], F32R),
        ("SELG", [DIN, NG * DIN], F32R), ("ONESG", [DIN, NG * DIN], BF16),
    ]:
        din[name] = nc.dram_tensor(name, shape, dt, kind="ExternalInput").ap()
    y_out = nc.dram_tensor("y_dir", [C, L], F32, kind="ExternalOutput").ap()

    def mm(ps, lhsT, rhs, start=True, stop=True):
        fd = rhs.shape[-1]
        for s0 in range(0, fd, 512):
            s1 = min(s0 + 512, fd)
            nc.tensor.matmul(ps[:, s0:s1], lhsT, rhs[:, s0:s1],
                             start=start, stop=stop)

    def mm_acc(ps, parts):
        fd = parts[0][1].shape[-1]
        for s0 in range(0, fd, 512):
            s1 = min(s0 + 512, fd)
            for k, (lhsT, rhs) in enumerate(parts):
                nc.tensor.matmul(ps[:, s0:s1], lhsT, rhs[:, s0:s1],
                                 start=(k == 0), stop=(k == len(parts) - 1))

    SL = 512
    NSL = LC // SL

    from contextlib import ExitStack
    with tile.TileContext(nc) as tc, ExitStack() as es:
        wp = es.enter_context(tc.tile_pool(name="wp", bufs=1))
        sp = es.enter_context(tc.tile_pool(name="sp", bufs=2))
        gp = es.enter_context(tc.tile_pool(name="gp", bufs=3))
        tp = es.enter_context(tc.tile_pool(name="tp", bufs=4))
        pp = es.enter_context(tc.tile_pool(name="pp", bufs=1))
        ap = es.enter_context(tc.tile_pool(name="ap", bufs=4))
        psG = es.enter_context(tc.tile_pool(name="psG", bufs=2, space="PSUM"))
        psY = es.enter_context(tc.tile_pool(name="psY", bufs=1, space="PSUM"))
        psR = es.enter_context(tc.tile_pool(name="psR", bufs=2, space="PSUM"))

        w = {}
        _first = ["SELG", "nin_wT", "s1", "t1", "w_in_x", "conv_diag",
                  "conv_b", "conv_bn"]
        _order = _first + [n for n in din if n not in _first]
        for name in _order:
            if name == "xb":
                continue
            t = wp.tile(list(din[name].shape), din[name].tensor.dtype,
                        name=f"w_{name}")
            nc.sync.dma_start(t, din[name])
            w[name] = t
        carry = wp.tile([DIN, NG], F32, name="carry")

        # ~5us of dense matmuls at NEFF start: ramps the PE HAM clock-gate
        # to 8/8 so the steady-state mms run at 2.4 GHz from the first chunk
        pswu = psG.tile([DIN, 512], F32, name="pswu", tag="psG")
        for _ in range(12):
            nc.tensor.matmul(pswu, w["SELG"][:, 0:DIN],
                             w["SELG"][:, 0:512], start=True, stop=True)

        # ---------------- pipelined emission ----------------
        # emit_gemm_stages(ch) returns 4 closures (S0..S3) that produce the
        # chunk's GEMM-phase tensors; they are interleaved into the PREVIOUS
        # chunk's group loop so PE/ACT/DVE queues stay dense across chunks.
        states = {}

        def emit_gemm_stages(ch):
            st = {}
            states[ch] = st
            lo = ch * LC

            def S0():
                x_t = sp.tile([C, LC], F32R, name="x_t", tag="x_t")
                nc.sync.dma_start(x_t, din["xb"][:, lo:lo + LC])
                act = sp.tile([C, LC], F32R, name="act", tag="act")
                for s in range(NSL):
                    sl = slice(s * SL, (s + 1) * SL)
                    ps = psG.tile([C, SL], F32, name="ps_h0", tag="psG")
                    nc.tensor.matmul(ps, w["nin_wT"], x_t[:, sl],
                                     start=True, stop=True)
                    nc.scalar.activation(act[:, sl], ps, AF.Relu,
                                         scale=w["s1"][:, 0:1],
                                         bias=w["t1"][:, 0:1])
                xmpre = sp.tile([DIN, LC + 3], F32R, name="xmpre", tag="xmpre")
                for s in range(NSL):
                    sl = slice(s * SL, (s + 1) * SL)
                    ps = psG.tile([DIN, SL], F32, name="ps_xx", tag="psG")
                    nc.tensor.matmul(ps, w["w_in_x"], act[:, sl],
                                     start=True, stop=True)
                    nc.scalar.copy(xmpre[:, 3 + s * SL:3 + (s + 1) * SL], ps)
                if ch == 0:
                    nc.vector.memset(xmpre[:, 0:3].bitcast(F32), 0.0)
                else:
                    prev = states[ch - 1]["xmpre"]
                    nc.scalar.copy(xmpre[:, 0:3], prev[:, LC:LC + 3])
                st["act"], st["xmpre"] = act, xmpre

            def S1():
                act, xmpre = st["act"], st["xmpre"]
                xm = sp.tile([DIN, LC], F32R, name="xm", tag="xm")
                for s in range(NSL):
                    sl = slice(s * SL, (s + 1) * SL)
                    psc = psG.tile([DIN, SL], F32, name="ps_xc", tag="psG")
                    for k in range(DCONV):
                        nc.tensor.matmul(
                            psc, w["conv_diag"][:, k * DIN:(k + 1) * DIN],
                            xmpre[:, k + s * SL:k + s * SL + SL],
                            start=(k == 0), stop=(k == DCONV - 1))
                    s1_ = tp.tile([DIN, SL], F32, name="s1c", tag="tmp")
                    nc.scalar.activation(s1_, psc, AF.Exp, scale=-1.0,
                                         bias=w["conv_bn"][:, 0:1])
                    s2_ = tp.tile([DIN, SL], F32, name="s2c", tag="tmp")
                    nc.scalar.activation(s2_, s1_, AF.Ln, bias=1.0)
                    sg_ = tp.tile([DIN, SL], F32, name="sgc", tag="tmp")
                    nc.scalar.activation(sg_, s2_, AF.Exp, scale=-1.0)
                    nc.vector.scalar_tensor_tensor(
                        xm[:, sl], psc, w["conv_b"][:, 0:1], sg_,
                        OP.add, OP.mult)
                st["xm"] = xm

            def S2():
                xm = st["xm"]
                NBC = DTR + 2 * N
                bcS = sp.tile([NBC, LC], F32R, name="bcS", tag="bcS")
                dwx = sp.tile([DIN, 2 * LC], F32R, name="dwx", tag="dwx")
                delta = dwx[:, 0:LC]
                for s in range(NSL):
                    sl = slice(s * SL, (s + 1) * SL)
                    psbc = psG.tile([NBC, SL], F32, name="ps_bc", tag="psG")
                    nc.tensor.matmul(psbc, w["w_xproj"], xm[:, sl],
                                     start=True, stop=True)
                    nc.scalar.copy(bcS[:, sl], psbc)
                for s in range(NSL):
                    sl = slice(s * SL, (s + 1) * SL)
                    psp = psG.tile([DIN, SL], F32, name="ps_dpre", tag="psG")
                    nc.tensor.matmul(psp, w["w_dt"], bcS[0:DTR, sl],
                                     start=True, stop=True)
                    e_ = tp.tile([DIN, SL], F32, name="e_", tag="tmp")
                    nc.scalar.activation(e_, psp, AF.Exp,
                                         bias=w["b_dt"][:, 0:1])
                    nc.scalar.activation(delta[:, sl], e_, AF.Ln, bias=1.0)
                nc.vector.tensor_tensor(dwx[:, LC:2 * LC], delta, xm, OP.mult)
                st["dwx"], st["bcS"] = dwx, bcS

            def S3():
                act = st["act"]
                zs = sp.tile([DIN, LC], F32, name="zs", tag="zs")
                for s in range(NSL):
                    sl = slice(s * SL, (s + 1) * SL)
                    psz = psG.tile([DIN, SL], F32, name="ps_z", tag="psG")
                    nc.tensor.matmul(psz, w["w_in_z"], act[:, sl],
                                     start=True, stop=True)
                    z1 = tp.tile([DIN, SL], F32, name="z1", tag="tmp")
                    nc.scalar.activation(z1, psz, AF.Exp, scale=-1.0)
                    z2 = tp.tile([DIN, SL], F32, name="z2", tag="tmp")
                    nc.scalar.activation(z2, z1, AF.Ln, bias=1.0)
                    z3 = tp.tile([DIN, SL], F32, name="z3", tag="tmp")
                    nc.scalar.activation(z3, z2, AF.Exp, scale=-1.0)
                    nc.vector.tensor_tensor(zs[:, sl], psz, z3, OP.mult)
                st["zs"] = zs

            return [S0, S1, S2, S3]

        def emit_bbcb(st):
            bcS = st["bcS"]
            psB = psR.tile([DIN, LC], F32, name="psB", tag="psR")
            mm(psB, w["SEL_B"], bcS)
            Bb = sp.tile([DIN, LC], BF16, name="Bb", tag="Bb")
            nc.scalar.copy(Bb, psB)
            psC = psR.tile([DIN, LC], F32, name="psC", tag="psR")
            mm(psC, w["SEL_C"], bcS)
            Cb = sp.tile([DIN, LC], BF16, name="Cb", tag="Cb")
            nc.scalar.copy(Cb, psC)
            st["Bb"], st["Cb"] = Bb, Cb

        # prologue: chunk 0 GEMM phase + Bb/Cb
        for f in emit_gemm_stages(0):
            f()
        emit_bbcb(states[0])

        for ch in range(NCH):
            st = states[ch]
            delta = st["dwx"][:, 0:LC]
            wdx = st["dwx"][:, LC:2 * LC]
            nxt = emit_gemm_stages(ch + 1) if ch + 1 < NCH else []
            psy = psY.tile([DIN, LC], F32, name="psy", tag="psy")
            # D * xm opens the accumulation group (xm is ready early);
            # the last ONESG matmul closes it, so y is unblocked as soon
            # as group 15's p lands.
            mm(psy, w["D_diag"], st["xm"], start=True, stop=False)
            p_tiles = {}
            for g in range(NG):
                sel_g = w["SELG"][:, g * DIN:(g + 1) * DIN]
                psdD = psR.tile([DIN, LC], F32, name="psdD", tag="psR")
                mm(psdD, sel_g, delta)
                dA = ap.tile([DIN, LC], F32, name="dA", tag="dA")
                nc.scalar.activation(dA, psdD, AF.Exp,
                                     scale=w["A_perm"][:, g:g + 1])
                psdW = psR.tile([DIN, LC], F32, name="psdW", tag="psR")
                mm(psdW, sel_g, wdx)
                wdxR = ap.tile([DIN, LC], BF16, name="wdxR", tag="wdxR")
                nc.scalar.copy(wdxR, psdW)
                dBu = ap.tile([DIN, LC], BF16, name="dBu", tag="dBu")
                nc.vector.tensor_tensor(dBu, wdxR, st["Bb"], OP.mult)
                h = gp.tile([DIN, LC], BF16, name="h", tag="h")
                init = 0.0 if ch == 0 else carry[:, g:g + 1]
                nc.vector.tensor_tensor_scan(h, dA, dBu, init,
                                             OP.mult, OP.add)
                nc.scalar.copy(carry[:, g:g + 1], h[:, LC - 1:LC])
                p = gp.tile([DIN, LC], BF16, name="p", tag="p")
                nc.vector.tensor_tensor(p, h, st["Cb"], OP.mult)
                p_tiles[g] = p
                # y-acc for the previous group (keeps PE stream dense)
                if g >= 1:
                    mm(psy, w["ONESG"][:, (g - 1) * DIN:g * DIN],
                       p_tiles.pop(g - 1), start=False, stop=False)
                # interleave next chunk's GEMM stages
                if nxt:
                    if g == 4:
                        nxt[0]()
                    elif g == 7:
                        nxt[1]()
                    elif g == 10:
                        nxt[2]()
                    elif g == 13:
                        nxt[3]()
            mm(psy, w["ONESG"][:, (NG - 1) * DIN:NG * DIN],
               p_tiles.pop(NG - 1), start=False, stop=True)

            # y = psy * zs ; project by w_out
            y = sp.tile([DIN, LC], F32R, name="y", tag="y")
            nc.vector.tensor_tensor(y, psy, st["zs"], OP.mult)
            yo = sp.tile([C, LC], F32, name="yo", tag="yo")
            for s in range(NSL):
                sl = slice(s * SL, (s + 1) * SL)
                pso = psG.tile([C, SL], F32, name="ps_yo", tag="psG")
                nc.tensor.matmul(pso, w["w_out_q"], y[:, sl],
                                 start=True, stop=True)
                nc.scalar.copy(yo[:, sl], pso)
            nc.sync.dma_start(y_out[:, ch * LC:(ch + 1) * LC], yo)
            if nxt:
                emit_bbcb(states[ch + 1])

    nc.compile()
    return nc


# ---------------------------------------------------------------- NEFF 2
def _build_neff2():
    nc = bacc.Bacc("TRN2", target_bir_lowering=False, debug=False,
                   num_devices=N_CORES)
    din = {}
    for name, shape, dt in [
        ("yq0", [C, LQ], F32), ("yq1", [C, LQ], F32), ("yq2", [C, LQ], F32),
        ("yq3", [C, LQ], F32),
        ("x_res", [C, LQ], F32R), ("nin_wT", [C, C], F32R),
        ("s1", [C, 1], F32), ("t1", [C, 1], F32),
        ("nin2_wT", [C, C], F32R), ("s2", [C, 1], F32), ("t2", [C, 1], F32),
    ]:
        din[name] = nc.dram_tensor(name, shape, dt, kind="ExternalInput").ap()
    o_out = nc.dram_tensor("out_q", [C, LQ], F32, kind="ExternalOutput").ap()

    LC2 = 512
    from contextlib import ExitStack
    with tile.TileContext(nc) as tc, ExitStack() as es:
        pool = es.enter_context(tc.tile_pool(name="p2", bufs=1))
        ch2 = es.enter_context(tc.tile_pool(name="ch2", bufs=3))
        psum = es.enter_context(tc.tile_pool(name="ps2", bufs=4, space="PSUM"))
        t = {}
        for name in ("nin_wT", "s1", "t1", "nin2_wT", "s2", "t2"):
            t[name] = pool.tile(list(din[name].shape),
                                din[name].tensor.dtype, name=f"t_{name}")
            nc.sync.dma_start(t[name], din[name])
        out_sb = pool.tile([C, LQ], F32, name="out_sb")
        for ci in range(LQ // LC2):
            sl = slice(ci * LC2, (ci + 1) * LC2)
            xs = ch2.tile([C, LC2], F32R, name="xs", tag="xs")
            nc.sync.dma_start(xs, din["x_res"][:, sl])
            yq = ch2.tile([C, 4, LC2], F32, name="yq", tag="yq")
            for q in range(4):
                nc.sync.dma_start(yq[:, q, :], din[f"yq{q}"][:, sl])
            ps = psum.tile([C, LC2], F32, name="ps_a", tag="ps2")
            nc.tensor.matmul(ps, t["nin_wT"], xs, start=True, stop=True)
            actq = ch2.tile([C, LC2], F32, name="actq", tag="actq")
            nc.scalar.activation(actq, ps, AF.Relu,
                                 scale=t["s1"][:, 0:1], bias=t["t1"][:, 0:1])
            a0 = ch2.tile([C, LC2], F32, name="a0", tag="a0")
            nc.vector.tensor_add(a0, yq[:, 0, :], yq[:, 1, :])
            a1 = ch2.tile([C, LC2], F32, name="a1", tag="a1")
            nc.vector.tensor_add(a1, yq[:, 2, :], yq[:, 3, :])
            a2 = ch2.tile([C, LC2], F32, name="a2", tag="a2")
            nc.vector.tensor_add(a2, a0, a1)
            pre = ch2.tile([C, LC2], F32R, name="pre", tag="pre")
            nc.vector.tensor_add(pre, a2, actq)
            ps2 = psum.tile([C, LC2], F32, name="ps_b", tag="ps2")
            nc.tensor.matmul(ps2, t["nin2_wT"], pre, start=True, stop=True)
            nc.scalar.activation(out_sb[:, sl], ps2, AF.Relu,
                                 scale=t["s2"][:, 0:1], bias=t["t2"][:, 0:1])
        nc.sync.dma_start(o_out, out_sb)
    nc.compile()
    return nc


# ---------------------------------------------------------------- runner
class _Cached:
    def __init__(self, nc):
        bass2jax.install_neuronx_cc_hook()
        self.nc = nc
        in_names, out_names, out_avals, zero_shapes = [], [], [], []
        pname = nc.partition_id_tensor.name if nc.partition_id_tensor else None
        for alloc in nc.m.functions[0].allocations:
            if not isinstance(alloc, mybir.MemoryLocationSet):
                continue
            name = alloc.memorylocations[0].name
            if alloc.kind == "ExternalInput":
                if name != pname:
                    in_names.append(name)
            elif alloc.kind == "ExternalOutput":
                out_names.append(name)
                shape = tuple(alloc.tensor_shape)
                dtype = mybir.dt.np(alloc.dtype)
                out_avals.append(jax.core.ShapedArray(shape, dtype))
                zero_shapes.append((shape, dtype))
        self.in_names, self.out_names = in_names, out_names
        self.out_avals, self.zero_shapes = out_avals, zero_shapes
        n_params, n_outs = len(in_names), len(out_names)
        all_in = list(in_names) + list(out_names)
        if pname is not None:
            all_in.append(pname)

        def _body(*args):
            operands = list(args)
            if pname is not None:
                operands.append(bass2jax.partition_id_tensor())
            return tuple(bass2jax._bass_exec_p.bind(
                *operands, out_avals=tuple(out_avals), in_names=tuple(all_in),
                out_names=tuple(out_names), lowering_input_output_aliases=(),
                sim_require_finite=True, sim_require_nnan=True, nc=nc))

        devices = jax.devices()[:N_CORES]
        mesh = Mesh(np.asarray(devices), ("core",))
        self.sharded = jax.jit(
            shard_map(_body, mesh=mesh,
                      in_specs=(PartitionSpec("core"),) * (n_params + n_outs),
                      out_specs=(PartitionSpec("core"),) * n_outs,
                      check_rep=False),
            donate_argnums=tuple(range(n_params, n_params + n_outs)),
            keep_unused=True)

    def run(self, in_maps):
        cc = [np.concatenate([np.ascontiguousarray(
                np.asarray(in_maps[c][nm], dtype=self._np_dtype(nm)))
              for c in range(N_CORES)], axis=0) for nm in self.in_names]
        zz = [np.zeros((N_CORES * s[0], *s[1:]), d)
              for (s, d) in self.zero_shapes]
        out = self.sharded(*cc, *zz)
        return [
            {nm: np.asarray(out[i]).reshape(N_CORES, *self.out_avals[i].shape)[c]
             for i, nm in enumerate(self.out_names)}
            for c in range(N_CORES)
        ]

    def _np_dtype(self, nm):
        for alloc in self.nc.m.functions[0].allocations:
            if (isinstance(alloc, mybir.MemoryLocationSet)
                    and alloc.memorylocations[0].name == nm):
                return mybir.dt.np(alloc.dtype)
        return np.float32


_CACHE = {}


def _get(key, builder):
    if key not in _CACHE:
        with _pinned_act_set():
            _CACHE[key] = _Cached(builder())
    return _CACHE[key]


# ---------------------------------------------------------------- host glue
def _selectors():
    """(n,ds) layout helpers.  partition index = n*DS + ds; d = g*DS + ds."""
    sel_b = np.zeros((DTR + 2 * N, DIN), np.float32)
    sel_c = np.zeros((DTR + 2 * N, DIN), np.float32)
    selg = np.zeros((DIN, NG * DIN), np.float32)
    onesg = np.zeros((DIN, NG * DIN), np.float32)
    for n in range(N):
        for ds in range(DS):
            prt = n * DS + ds
            sel_b[DTR + n, prt] = 1.0
            sel_c[DTR + N + n, prt] = 1.0
            for g in range(NG):
                d = g * DS + ds
                selg[d, g * DIN + prt] = 1.0
                onesg[prt, g * DIN + d] = 1.0
    return sel_b, sel_c, selg, onesg


def kernel(**inputs):
    x = np.asarray(inputs["x"], np.float32).reshape(B, C, L)
    s1 = (np.asarray(inputs["g1"]) / np.sqrt(np.asarray(inputs["v1"]) + BN_EPS)
          ).astype(np.float32)
    t1 = (np.asarray(inputs["b1"]) - np.asarray(inputs["m1"]) * s1
          ).astype(np.float32)
    s2 = (np.asarray(inputs["g2"]) / np.sqrt(np.asarray(inputs["v2"]) + BN_EPS)
          ).astype(np.float32)
    t2 = (np.asarray(inputs["b2"]) - np.asarray(inputs["m2"]) * s2
          ).astype(np.float32)
    w_in = np.asarray(inputs["w_in"], np.float32)
    w_out = np.asarray(inputs["w_out"], np.float32)
    conv_w = np.asarray(inputs["conv_w"], np.float32)
    conv_b = np.asarray(inputs["conv_b"], np.float32)
    A_neg = (-np.exp(np.asarray(inputs["A_log"]))).astype(np.float32)
    D_param = np.asarray(inputs["D_param"], np.float32)
    nin_wT = np.ascontiguousarray(np.asarray(inputs["nin_w"], np.float32).T)
    nin2_wT = np.ascontiguousarray(np.asarray(inputs["nin2_w"], np.float32).T)
    conv_diag = np.zeros((DIN, DCONV * DIN), np.float32)
    for k in range(DCONV):
        conv_diag[:, k * DIN:(k + 1) * DIN][np.arange(DIN), np.arange(DIN)] = \
            conv_w[:, k]
    # A_perm[(n,ds), g] = A_neg[g*DS+ds, n]
    A_perm = np.zeros((DIN, NG), np.float32)
    for n in range(N):
        for ds in range(DS):
            A_perm[n * DS + ds, :] = A_neg[np.arange(NG) * DS + ds, n]
    sel_b, sel_c, selg, onesg = _selectors()

    k1 = _get("n1", _build_neff1)
    k2 = _get("n2", _build_neff2)

    com = dict(
        nin_wT=nin_wT, s1=s1[:, None], t1=t1[:, None],
        conv_diag=conv_diag, conv_b=conv_b[:, None],
        conv_bn=(-conv_b)[:, None],
        w_xproj=np.asarray(inputs["w_xproj"], np.float32),
        w_dt=np.asarray(inputs["w_dt"], np.float32),
        b_dt=np.asarray(inputs["b_dt"], np.float32)[:, None],
        A_perm=A_perm, D_diag=np.diag(D_param).astype(np.float32),
        SEL_B=sel_b, SEL_C=sel_c, SELG=selg, ONESG=onesg.astype(ml_bf16),
    )
    in1 = []
    for core in range(N_CORES):
        b, d = core // 4, core % 4
        cflip, lflip = d in (1, 3), d in (2, 3)
        wi = w_in[::-1].copy() if cflip else w_in
        wo = (w_out[:, ::-1].copy() if cflip else w_out) / 4.0
        xb = x[b][:, ::-1].copy() if lflip else x[b]
        m = dict(com)
        m.update(xb=xb, w_in_x=np.ascontiguousarray(wi[:, :DIN]),
                 w_in_z=np.ascontiguousarray(wi[:, DIN:]),
                 w_out_q=np.ascontiguousarray(wo))
        in1.append(m)
    res1 = k1.run(in1)

    ys = []
    for core in range(N_CORES):
        y = res1[core]["y_dir"]
        if core % 4 in (2, 3):
            y = y[:, ::-1]
        ys.append(y)

    in2 = []
    for core in range(N_CORES):
        b, q = core // 4, core % 4
        sl = slice(q * LQ, (q + 1) * LQ)
        m = dict(
            yq0=np.ascontiguousarray(ys[b * 4 + 0][:, sl]),
            yq1=np.ascontiguousarray(ys[b * 4 + 1][:, sl]),
            yq2=np.ascontiguousarray(ys[b * 4 + 2][:, sl]),
            yq3=np.ascontiguousarray(ys[b * 4 + 3][:, sl]),
            x_res=np.ascontiguousarray(x[b][:, sl]),
            nin_wT=nin_wT, s1=s1[:, None], t1=t1[:, None],
            nin2_wT=nin2_wT, s2=s2[:, None], t2=t2[:, None],
        )
        in2.append(m)
    res2 = k2.run(in2)

    out = np.zeros((B, C, L), np.float32)
    for core in range(N_CORES):
        b, q = core // 4, core % 4
        out[b, :, q * LQ:(q + 1) * LQ] = res2[core]["out_q"]
    return out.reshape(B, C, DZ, H, W)
